# revision 43
# baseline (speedup 1.0000x reference)
"""Trainium2 Bass kernel for nn_Attention_13348758356565.

Dense transformer attention block (B=16, N=1024 tokens, DIM=1024, 16 heads x 64)
with axial rotary embeddings, data-parallel over batch across 8 NeuronCores
(2 batches per core). v2 design:

- All-bf16 data path (verified ~7e-3 absmax-rel on CPU sim vs 2e-2 gate):
  x, w_qkv, q/k/v, attention probs, w_proj, y all bf16; PSUM accumulation and
  softmax denominators fp32.
- Token-stationary QKV projection; q/k head dims de-interleaved host-side
  (evens|odds|pass per head) so rotary runs as dense step-1 bf16 DVE ops.
- q/k transposed to [feature, token] layout with DMA xbar transposes
  (dma_start_transpose) instead of PE transposes: PE transposes don't count
  as HAM activity and were keeping the PE clock-gate cold (K=4/8) for 70%
  of the baseline span.
- Attention per head-pair with 2-head row-group packing (K=64 concurrent
  matmul pairs), fused [128,1024] exp on ScalarE, softmax denominators from
  an appended ones-column in the PV stationary operand, nch-outer loop so
  PSUM fits exactly in 8 banks (2x st2@2 + 2x pv@1 + 2x pq@1).
- Emission-level software pipelining: QKV/proj matmul streams of the next
  group/batch are interleaved into the attention phases (engine queues are
  FIFO) so the PE never idles long enough for the HAM clock gate to
  re-throttle.
"""

import os
import sys

sys.path.insert(0, "/opt/trn_rl_repo")

import dataclasses
import numpy as np

import concourse.bacc as bacc
import concourse.mybir as mybir
import concourse.tile as tile
from concourse import bass_utils

F32 = mybir.dt.float32
BF16 = mybir.dt.bfloat16
EXP = mybir.ActivationFunctionType.Exp

B, HF, WF = 16, 32, 32
DIM, NH, HD = 1024, 16, 64
N = HF * WF          # 1024 tokens
NCORES = 8
BPC = B // NCORES    # 2 batches per core
ROT = HD // 2        # 32 rotary dims per head
SCALE = 1.0 / np.sqrt(HD)
NT = N // 128        # 8 token tiles
ND = DIM // 128      # 8 contraction tiles
HP = NH // 2         # 8 head pairs

last_exec_time_ns = None

_SENTINEL = object()


def _bcast_mid(ap, count):
    """Insert a step-0 (broadcast) middle dim into a [P, C] AP -> [P, count, C]."""
    return dataclasses.replace(ap, ap=[ap.ap[0], [0, count], ap.ap[1]])


def _freq_tables():
    d = HD // 4
    base = (np.linspace(1.0, (HF * WF) / 2.0, d // 2, dtype=np.float64) * np.pi)
    posH = np.linspace(-1.0, 1.0, HF)
    posW = np.linspace(-1.0, 1.0, WF)
    fH = np.repeat(posH[:, None] * base[None, :], 2, axis=-1)   # [H, 16]
    fW = np.repeat(posW[:, None] * base[None, :], 2, axis=-1)   # [W, 16]
    fH = np.broadcast_to(fH[:, None, :], (HF, WF, d))
    fW = np.broadcast_to(fW[None, :, :], (HF, WF, d))
    freqs = np.concatenate([fH, fW], axis=-1).reshape(N, ROT)
    # freqs[:, 2i] == freqs[:, 2i+1]; keep one per pair -> [N, 16]
    half = freqs[:, 0::2]
    return np.cos(half), np.sin(half)


def _drain(it, n):
    for _ in range(n):
        if next(it, _SENTINEL) is _SENTINEL:
            return


def _drain_all(it):
    for _ in it:
        pass


def _chain(*its):
    for it in its:
        yield from it


def _build():
    nc = bacc.Bacc("TRN2", target_bir_lowering=False, debug=False)

    xT_d = nc.dram_tensor("xT", [BPC, DIM, N], BF16, kind="ExternalInput")
    wqkvT_d = nc.dram_tensor("wqkvT", [DIM, 3 * DIM], BF16, kind="ExternalInput")
    wprojT_d = nc.dram_tensor("wprojT", [DIM, DIM], BF16, kind="ExternalInput")
    cosh_d = nc.dram_tensor("cosh", [N, 16], BF16, kind="ExternalInput")
    sinh_d = nc.dram_tensor("sinh", [N, 16], BF16, kind="ExternalInput")
    biasbc_d = nc.dram_tensor("biasbc", [128, DIM], BF16, kind="ExternalInput")
    y_d = nc.dram_tensor("y", [BPC, N, DIM], BF16, kind="ExternalOutput")

    mul = mybir.AluOpType.mult
    sub = mybir.AluOpType.subtract
    add = mybir.AluOpType.add

    with tile.TileContext(nc) as tc:
        with (
            tc.tile_pool(name="sb", bufs=1) as sb,
            tc.tile_pool(name="ps", bufs=1, space="PSUM") as ps,
        ):
            # ---- constants / weights (resident; off the critical sync queue) ----
            biasBC = sb.tile([128, DIM], BF16, name="biasBC")
            cosh = sb.tile([128, NT * 16], BF16, name="cosh")
            sinh = sb.tile([128, NT * 16], BF16, name="sinh")
            nc.scalar.dma_start(
                cosh[:].rearrange("p (t c) -> p t c", c=16),
                cosh_d.ap().rearrange("(t p) c -> p t c", p=128),
            )
            nc.scalar.dma_start(
                sinh[:].rearrange("p (t c) -> p t c", c=16),
                sinh_d.ap().rearrange("(t p) c -> p t c", p=128),
            )
            wprojT = [sb.tile([128, DIM], BF16, name=f"wprojT{d}") for d in range(ND)]

            def load_wproj():
                nc.scalar.dma_start(biasBC[:], biasbc_d.ap())
                for d in range(ND):
                    nc.sync.dma_start(wprojT[d][:],
                                      wprojT_d.ap()[d * 128:(d + 1) * 128, :])

            # per-batch persistent tiles (slot-rotated via tags, bufs=2)
            xT = {}      # b -> [128, ND*N] (col block d*N..: contraction rows d*128..)
            vsb = {}     # b -> [NT] tiles [128, NH*(HD+1)]
            outT = {}    # b -> [ND] tiles [128, N]
            qTg = {}     # (b, g) -> [128, 4*N]
            kTg = {}     # (b, g) -> [128, 4*N]

            def alloc_x(b, ds=None):
                if b not in xT:
                    xT[b] = [sb.tile([128, N], BF16, name=f"xT_b{b}_{d}",
                                     tag=f"xT{d}", bufs=2) for d in range(ND)]
                for d in (range(ND) if ds is None else ds):
                    eng = nc.sync if d % 2 == 0 else nc.scalar
                    eng.dma_start(xT[b][d][:],
                                  xT_d.ap()[b, d * 128:(d + 1) * 128, :])

            def alloc_v(b):
                vsb[b] = []
                for t in range(NT):
                    v = sb.tile([128, NH * (HD + 1)], BF16, name=f"v_b{b}_{t}",
                                tag=f"v{t}", bufs=2)
                    nc.vector.memset(
                        v[:].rearrange("p (h c) -> p h c", c=HD + 1)[:, :, HD:], 1.0)
                    vsb[b].append(v)

            def alloc_out(b):
                outT[b] = [sb.tile([128, N], BF16, name=f"outT_b{b}_{d}",
                                   tag=f"outT{d}", bufs=2) for d in range(ND)]

            def alloc_qk(b, g):
                qTg[(b, g)] = sb.tile([128, 4 * N], BF16, name=f"qT_b{b}_g{g}",
                                      tag="qTg", bufs=2)
                kTg[(b, g)] = sb.tile([128, 4 * N], BF16, name=f"kT_b{b}_g{g}",
                                      tag="kTg", bufs=2)

            # ---------------- QKV generators (token-stationary) -------------
            def eat_v(b, jc):
                def eat(t, pq):
                    h0 = jc * 8
                    nc.vector.tensor_copy(
                        vsb[b][t][:].rearrange("p (h c) -> p h c", c=HD + 1)
                        [:, h0:h0 + 8, 0:HD],
                        pq.rearrange("p (h c) -> p h c", c=HD))
                return eat

            def eat_qk(b, g, dst):
                def eat(t, pq):
                    qn = sb.tile([128, 512], BF16, name=f"qn_{b}_{g}_{t}",
                                 tag="qn", bufs=6)
                    nc.vector.tensor_copy(qn[:], pq)   # cast + free psum
                    on = qn[:].rearrange("p (h u) -> p h u", u=HD)
                    ev, od = on[:, :, 0:16], on[:, :, 16:32]
                    cb = _bcast_mid(cosh[:, t * 16:(t + 1) * 16], 8)
                    sbb = _bcast_mid(sinh[:, t * 16:(t + 1) * 16], 8)
                    ts4 = []
                    for i in range(4):
                        ti = sb.tile([128, 8, 16], BF16, name=f"t{i}_{b}_{g}_{t}",
                                     tag=f"rtmp{i}", bufs=2)
                        ts4.append(ti)
                    nc.vector.tensor_tensor(ts4[0][:], ev, cb, mul)
                    nc.vector.tensor_tensor(ts4[1][:], od, sbb, mul)
                    nc.vector.tensor_tensor(ts4[2][:], od, cb, mul)
                    nc.vector.tensor_tensor(ts4[3][:], ev, sbb, mul)
                    nc.vector.tensor_tensor(ev, ts4[0][:], ts4[1][:], sub)
                    nc.vector.tensor_tensor(od, ts4[2][:], ts4[3][:], add)
                    # transpose [128 tok, 512 feat] -> 4 x [128 feat, 128 tok]
                    dst_view = (dst[:].rearrange("p (j c) -> p j c", c=N)
                                [:, :, t * 128:(t + 1) * 128])
                    nc.sync.dma_start_transpose(dst_view, qn[:])
                return eat

            def gen_qkv(b, j0, consume, deep=False):
                """512-wide qkv chunk, one yield per matmul (64 total).
                deep=True borrows the idle st2 tag as a 4-deep psum rotation
                (pure phases only -- st2 belongs to attention otherwise)."""
                wts = []
                for h in range(4):
                    w = sb.tile([128, 1024], BF16, name=f"wq_{b}_{j0}_{h}",
                                tag=f"wqh{h}", bufs=2)
                    eng = nc.sync if h % 2 == 0 else nc.scalar
                    eng.dma_start(
                        w[:].rearrange("p (d c) -> p d c", c=512),
                        wqkvT_d.ap()[h * 256:(h + 1) * 256, j0:j0 + 512]
                        .rearrange("(d p) c -> p d c", p=128))
                    wts.append(w)
                big = None
                for t in range(NT):
                    if deep:
                        if t % 2 == 0:
                            big = ps.tile([128, 1024], F32,
                                          name=f"pqd_{b}_{j0}_{t}",
                                          tag="st2", bufs=2)
                        pq = big[:][:, (t % 2) * 512:(t % 2) * 512 + 512]
                    else:
                        pq = ps.tile([128, 512], F32, name=f"pq_{b}_{j0}_{t}",
                                     tag="pq", bufs=2)[:]
                    for d in range(ND):
                        nc.tensor.matmul(
                            pq, xT[b][d][:, t * 128:(t + 1) * 128],
                            wts[d // 2][:, (d % 2) * 512:(d % 2 + 1) * 512],
                            start=(d == 0), stop=(d == ND - 1))
                        if d == ND - 1:
                            consume(t, pq)
                        yield

            def gen_proj(b):
                for t in range(NT):
                    for ec in range(2):
                        py = ps.tile([128, 512], F32, name=f"py_{b}_{t}_{ec}",
                                     tag="pq", bufs=2)
                        for d in range(ND):
                            nc.tensor.matmul(
                                py[:], outT[b][d][:, t * 128:(t + 1) * 128],
                                wprojT[d][:, ec * 512:(ec + 1) * 512],
                                start=(d == 0), stop=(d == ND - 1))
                            yield
                        ysb = sb.tile([128, 512], BF16, name=f"y_{b}_{t}_{ec}",
                                      tag="ysb", bufs=3)
                        nc.vector.tensor_tensor(
                            ysb[:], py[:],
                            biasBC[:, ec * 512:(ec + 1) * 512], add)
                        nc.sync.dma_start(
                            y_d.ap()[b, t * 128:(t + 1) * 128,
                                     ec * 512:(ec + 1) * 512], ysb[:])
                        yield

            # ---------------- attention ----------------
            def attention(b, g, filler, quota):
                credit = 0.0

                def fill():
                    nonlocal credit
                    credit += quota
                    k = int(credit)
                    credit -= k
                    _drain(filler, k)

                qt, kt = qTg[(b, g)], kTg[(b, g)]
                for hl in range(4):
                    hp = 4 * g + hl
                    for nch in range(2):
                        pvs = [ps.tile([HD + 1, 512], F32,
                                       name=f"pv_{b}_{hp}_{nch}_{i}",
                                       tag="pv", bufs=2) for i in range(2)]
                        pts = {}
                        for m in range(NT + 2):
                            if m < NT:
                                st2 = ps.tile([128, 1024], F32,
                                              name=f"st_{b}_{hp}_{nch}_{m}",
                                              tag="st2", bufs=2)
                                for half in range(2):
                                    r0 = half * 64
                                    nc.tensor.matmul(
                                        st2[:, half * 512:(half + 1) * 512],
                                        kt[r0:r0 + 64,
                                           hl * N + m * 128: hl * N + (m + 1) * 128],
                                        qt[r0:r0 + 64,
                                           hl * N + nch * 512: hl * N + nch * 512 + 512])
                                pt2 = sb.tile([128, 1024], BF16,
                                              name=f"pt_{b}_{hp}_{nch}_{m}",
                                              tag="pt2", bufs=5)
                                nc.scalar.activation(pt2[:], st2[:], EXP,
                                                     scale=float(SCALE))
                                pts[m] = pt2
                            fill()
                            if m > 1:
                                pm = m - 2
                                for half in range(2):
                                    h = hp * 2 + half
                                    nc.tensor.matmul(
                                        pvs[half][:],
                                        vsb[b][pm][:, h * (HD + 1):(h + 1) * (HD + 1)],
                                        pts[pm][:, half * 512:(half + 1) * 512],
                                        start=(pm == 0), stop=(pm == NT - 1))
                                del pts[pm]
                            fill()
                        # normalize into outT
                        for half in range(2):
                            p = pvs[half]
                            dr = sb.tile([1, 512], F32,
                                         name=f"dr_{b}_{hp}_{nch}_{half}",
                                         tag="dr", bufs=2)
                            nc.vector.tensor_copy(dr[:], p[64:65, :])
                            rr = sb.tile([1, 512], F32,
                                         name=f"rr_{b}_{hp}_{nch}_{half}",
                                         tag="rr", bufs=2)
                            nc.vector.reciprocal_approx_fast(rr[:], dr[:])
                            rb = sb.tile([64, 512], F32,
                                         name=f"rb_{b}_{hp}_{nch}_{half}",
                                         tag="rb", bufs=2)
                            nc.gpsimd.partition_broadcast(rb[:], rr[:])
                            nc.vector.tensor_tensor(
                                outT[b][hp][half * 64:half * 64 + 64,
                                            nch * 512:(nch + 1) * 512],
                                p[0:64, :], rb[:], mul)

            # ---------------- schedule ----------------
            QS, KS, VS = 0, DIM, 2 * DIM

            # Phase A: b0 V + b0 g0 K,Q (pure PE streams; warms HAM)
            # DMA order matters: per-queue transfers serialize, so emit only
            # xT d0/d1 before the first chunk's weight loads.
            alloc_x(0, ds=[0, 1])
            alloc_v(0)
            alloc_out(0)
            gv0 = gen_qkv(0, VS, eat_v(0, 0), deep=True)
            _drain(gv0, 1)          # emits the chunk's weight DMAs + first MM
            alloc_x(0, ds=[2, 3, 4, 5, 6, 7])
            _drain_all(gv0)
            alloc_qk(0, 0)
            _drain_all(gen_qkv(0, KS + 0 * 512, eat_qk(0, 0, kTg[(0, 0)]),
                               deep=True))
            _drain_all(gen_qkv(0, QS + 0 * 512, eat_qk(0, 0, qTg[(0, 0)]),
                               deep=True))

            # Phase B: attn(b0,g0) + filler K,Q(b0,g1)
            alloc_x(1)
            alloc_qk(0, 1)
            fB = _chain(gen_qkv(0, KS + 512, eat_qk(0, 1, kTg[(0, 1)])),
                        gen_qkv(0, QS + 512, eat_qk(0, 1, qTg[(0, 1)])),
                        gen_qkv(0, VS + 512, eat_v(0, 1)))
            attention(0, 0, fB, 192 / 160)
            _drain_all(fB)

            # Phase C: attn(b0,g1) + filler V(b1), K,Q(b1,g0)
            load_wproj()
            alloc_v(1)
            alloc_out(1)
            alloc_qk(1, 0)
            fC = _chain(gen_qkv(1, VS, eat_v(1, 0)),
                        gen_qkv(1, KS, eat_qk(1, 0, kTg[(1, 0)])),
                        gen_qkv(1, QS, eat_qk(1, 0, qTg[(1, 0)])))
            attention(0, 1, fC, 192 / 160)
            _drain_all(fC)

            # Phase D: attn(b1,g0) + filler K,Q(b1,g1)
            alloc_qk(1, 1)
            fD = _chain(gen_qkv(1, KS + 512, eat_qk(1, 1, kTg[(1, 1)])),
                        gen_qkv(1, QS + 512, eat_qk(1, 1, qTg[(1, 1)])),
                        gen_qkv(1, VS + 512, eat_v(1, 1)))
            attention(1, 0, fD, 192 / 160)
            _drain_all(fD)

            # Phase E: attn(b1,g1) + filler proj(b0)
            fE = gen_proj(0)
            attention(1, 1, fE, 144 / 160)
            _drain_all(fE)

            # Phase F: proj(b1)
            _drain_all(gen_proj(1))

    nc.compile()
    return nc


_NC_CACHE = None


def _head_perm():
    """Per-head de-interleave: [evens of rot, odds of rot, pass dims]."""
    p = list(range(0, ROT, 2)) + list(range(1, ROT, 2)) + list(range(ROT, HD))
    return np.array(p)


def kernel(x, w_qkv, w_proj, b_proj):
    global _NC_CACHE, last_exec_time_ns
    import ml_dtypes
    bf = ml_dtypes.bfloat16
    x = np.ascontiguousarray(np.asarray(x, np.float32))
    w_qkv = np.asarray(w_qkv, np.float32)
    w_proj = np.asarray(w_proj, np.float32)
    b_proj = np.asarray(b_proj, np.float32)

    if _NC_CACHE is None:
        _NC_CACHE = _build()
    nc = _NC_CACHE

    cos_h, sin_h = _freq_tables()
    # reorder q/k rows: per head [evens|odds|pass]
    perm = _head_perm()
    full_perm = np.concatenate([h * HD + perm for h in range(NH)])
    wq = w_qkv[:DIM][full_perm]
    wk = w_qkv[DIM:2 * DIM][full_perm]
    wv = w_qkv[2 * DIM:]
    wqkvT = np.ascontiguousarray(
        np.concatenate([wq, wk, wv], axis=0).T).astype(bf)
    wprojT16 = np.ascontiguousarray(w_proj.T).astype(bf)
    biasbc16 = np.ascontiguousarray(
        np.broadcast_to(b_proj.reshape(1, DIM), (128, DIM))).astype(bf)
    cos16 = cos_h.astype(bf)
    sin16 = sin_h.astype(bf)

    in_maps = []
    for c in range(NCORES):
        xs = x[c * BPC:(c + 1) * BPC]                       # [2, N, DIM]
        xT = np.ascontiguousarray(xs.transpose(0, 2, 1)).astype(bf)
        in_maps.append({
            "xT": xT, "wqkvT": wqkvT, "wprojT": wprojT16,
            "biasbc": biasbc16, "cosh": cos16, "sinh": sin16,
        })

    trace = bool(os.environ.get("KERNEL_TRACE"))
    kwargs = {}
    if trace:
        kwargs["trace"] = True
        td = os.environ.get("KERNEL_TRACE_DIR")
        if td:
            kwargs["tmpdir"] = td
    res = bass_utils.run_bass_kernel_spmd(
        nc, in_maps, core_ids=list(range(NCORES)), **kwargs)
    last_exec_time_ns = res.exec_time_ns
    out = np.concatenate(
        [np.asarray(res.results[c]["y"]) for c in range(NCORES)], axis=0)
    return np.ascontiguousarray(
        out.reshape(B, N, DIM).astype(np.float32))


if __name__ == "__main__":
    rng = np.random.default_rng(0)
    xs = rng.standard_normal((B, N, DIM), dtype=np.float32)
    wq = rng.standard_normal((3 * DIM, DIM), dtype=np.float32) / 32
    wp = rng.standard_normal((DIM, DIM), dtype=np.float32) / 32
    bp = np.zeros(DIM, np.float32)
    y = kernel(xs, wq, wp, bp)
    print("y", y.shape, y.dtype, float(np.abs(y).max()))


# revision 45
# speedup vs baseline: 1.0524x; 1.0524x over previous
"""Trainium2 Bass kernel for nn_Attention_13348758356565.

Dense transformer attention block (B=16, N=1024 tokens, DIM=1024, 16 heads x 64)
with axial rotary embeddings, data-parallel over batch across 8 NeuronCores
(2 batches per core). v2 design:

- All-bf16 data path (verified ~7e-3 absmax-rel on CPU sim vs 2e-2 gate):
  x, w_qkv, q/k/v, attention probs, w_proj, y all bf16; PSUM accumulation and
  softmax denominators fp32.
- Token-stationary QKV projection; q/k head dims de-interleaved host-side
  (evens|odds|pass per head) so rotary runs as dense step-1 bf16 DVE ops.
- q/k transposed to [feature, token] layout with DMA xbar transposes
  (dma_start_transpose) instead of PE transposes: PE transposes don't count
  as HAM activity and were keeping the PE clock-gate cold (K=4/8) for 70%
  of the baseline span.
- Attention per head-pair with 2-head row-group packing (K=64 concurrent
  matmul pairs), fused [128,1024] exp on ScalarE, softmax denominators from
  an appended ones-column in the PV stationary operand, nch-outer loop so
  PSUM fits exactly in 8 banks (2x st2@2 + 2x pv@1 + 2x pq@1).
- Emission-level software pipelining: QKV/proj matmul streams of the next
  group/batch are interleaved into the attention phases (engine queues are
  FIFO) so the PE never idles long enough for the HAM clock gate to
  re-throttle.
"""

import os
import sys

sys.path.insert(0, "/opt/trn_rl_repo")

import dataclasses
import numpy as np

import concourse.bacc as bacc
import concourse.mybir as mybir
import concourse.tile as tile
from concourse import bass_utils

F32 = mybir.dt.float32
BF16 = mybir.dt.bfloat16
EXP = mybir.ActivationFunctionType.Exp

B, HF, WF = 16, 32, 32
DIM, NH, HD = 1024, 16, 64
N = HF * WF          # 1024 tokens
NCORES = 8
BPC = B // NCORES    # 2 batches per core
ROT = HD // 2        # 32 rotary dims per head
SCALE = 1.0 / np.sqrt(HD)
NT = N // 128        # 8 token tiles
ND = DIM // 128      # 8 contraction tiles
HP = NH // 2         # 8 head pairs

last_exec_time_ns = None

_SENTINEL = object()


def _bcast_mid(ap, count):
    """Insert a step-0 (broadcast) middle dim into a [P, C] AP -> [P, count, C]."""
    return dataclasses.replace(ap, ap=[ap.ap[0], [0, count], ap.ap[1]])


def _freq_tables():
    d = HD // 4
    base = (np.linspace(1.0, (HF * WF) / 2.0, d // 2, dtype=np.float64) * np.pi)
    posH = np.linspace(-1.0, 1.0, HF)
    posW = np.linspace(-1.0, 1.0, WF)
    fH = np.repeat(posH[:, None] * base[None, :], 2, axis=-1)   # [H, 16]
    fW = np.repeat(posW[:, None] * base[None, :], 2, axis=-1)   # [W, 16]
    fH = np.broadcast_to(fH[:, None, :], (HF, WF, d))
    fW = np.broadcast_to(fW[None, :, :], (HF, WF, d))
    freqs = np.concatenate([fH, fW], axis=-1).reshape(N, ROT)
    # freqs[:, 2i] == freqs[:, 2i+1]; keep one per pair -> [N, 16]
    half = freqs[:, 0::2]
    return np.cos(half), np.sin(half)


def _drain(it, n):
    for _ in range(n):
        if next(it, _SENTINEL) is _SENTINEL:
            return


def _drain_all(it):
    for _ in it:
        pass


def _chain(*its):
    for it in its:
        yield from it


HAS_BIAS = True


def _build(has_bias=True):
    global HAS_BIAS
    HAS_BIAS = has_bias
    nc = bacc.Bacc("TRN2", target_bir_lowering=False, debug=False)

    xT_d = nc.dram_tensor("xT", [BPC, DIM, N], BF16, kind="ExternalInput")
    wqkvT_d = nc.dram_tensor("wqkvT", [DIM, 3 * DIM], BF16, kind="ExternalInput")
    wprojT_d = nc.dram_tensor("wprojT", [DIM, DIM], BF16, kind="ExternalInput")
    cosh_d = nc.dram_tensor("cosh", [N, 16], BF16, kind="ExternalInput")
    sinh_d = nc.dram_tensor("sinh", [N, 16], BF16, kind="ExternalInput")
    biasbc_d = nc.dram_tensor("biasbc", [128, DIM], BF16, kind="ExternalInput")
    y_d = nc.dram_tensor("y", [BPC, N, DIM], BF16, kind="ExternalOutput")

    mul = mybir.AluOpType.mult
    sub = mybir.AluOpType.subtract
    add = mybir.AluOpType.add

    with tile.TileContext(nc) as tc:
        with (
            tc.tile_pool(name="sb", bufs=1) as sb,
            tc.tile_pool(name="ps", bufs=1, space="PSUM") as ps,
        ):
            # ---- constants / weights (resident; off the critical sync queue) ----
            biasBC = sb.tile([128, DIM], BF16, name="biasBC")
            nc.scalar.dma_start(biasBC[:], biasbc_d.ap())
            cosh = sb.tile([128, NT * 16], BF16, name="cosh")
            sinh = sb.tile([128, NT * 16], BF16, name="sinh")
            nc.scalar.dma_start(
                cosh[:].rearrange("p (t c) -> p t c", c=16),
                cosh_d.ap().rearrange("(t p) c -> p t c", p=128),
            )
            nc.scalar.dma_start(
                sinh[:].rearrange("p (t c) -> p t c", c=16),
                sinh_d.ap().rearrange("(t p) c -> p t c", p=128),
            )
            wprojT = [sb.tile([128, DIM], BF16, name=f"wprojT{d}") for d in range(ND)]

            def load_wproj():
                for d in range(ND):
                    nc.sync.dma_start(wprojT[d][:],
                                      wprojT_d.ap()[d * 128:(d + 1) * 128, :])

            # per-batch persistent tiles (slot-rotated via tags, bufs=2)
            xT = {}      # b -> [128, ND*N] (col block d*N..: contraction rows d*128..)
            vsb = {}     # b -> [NT] tiles [128, NH*(HD+1)]
            outT = {}    # b -> [ND] tiles [128, N]
            qTg = {}     # (b, g) -> [128, 4*N]
            kTg = {}     # (b, g) -> [128, 4*N]

            def alloc_x(b, ds=None):
                if b not in xT:
                    xT[b] = [sb.tile([128, N], BF16, name=f"xT_b{b}_{d}",
                                     tag=f"xT{d}", bufs=2) for d in range(ND)]
                for d in (range(ND) if ds is None else ds):
                    eng = nc.sync if d % 2 == 0 else nc.scalar
                    eng.dma_start(xT[b][d][:],
                                  xT_d.ap()[b, d * 128:(d + 1) * 128, :])

            def alloc_v(b):
                vsb[b] = []
                for t in range(NT):
                    v = sb.tile([128, NH * (HD + 1)], BF16, name=f"v_b{b}_{t}",
                                tag=f"v{t}", bufs=2)
                    nc.vector.memset(
                        v[:].rearrange("p (h c) -> p h c", c=HD + 1)[:, :, HD:], 1.0)
                    vsb[b].append(v)

            def alloc_out(b):
                outT[b] = [sb.tile([128, N], BF16, name=f"outT_b{b}_{d}",
                                   tag=f"outT{d}", bufs=2) for d in range(ND)]

            def alloc_qk(b, g):
                qTg[(b, g)] = sb.tile([128, 4 * N], BF16, name=f"qT_b{b}_g{g}",
                                      tag="qTg", bufs=2)
                kTg[(b, g)] = sb.tile([128, 4 * N], BF16, name=f"kT_b{b}_g{g}",
                                      tag="kTg", bufs=2)

            # ---------------- QKV generators (token-stationary) -------------
            def eat_v(b, jc):
                def eat(t, pq):
                    h0 = jc * 8
                    nc.vector.tensor_copy(
                        vsb[b][t][:].rearrange("p (h c) -> p h c", c=HD + 1)
                        [:, h0:h0 + 8, 0:HD],
                        pq[:].rearrange("p (h c) -> p h c", c=HD))
                return eat

            def eat_qk(b, g, dst):
                def eat(t, pq):
                    qn = sb.tile([128, 512], BF16, name=f"qn_{b}_{g}_{t}",
                                 tag="qn", bufs=5)
                    nc.vector.tensor_copy(qn[:], pq[:])   # cast + free psum
                    on = qn[:].rearrange("p (h u) -> p h u", u=HD)
                    ev, od = on[:, :, 0:16], on[:, :, 16:32]
                    cb = _bcast_mid(cosh[:, t * 16:(t + 1) * 16], 8)
                    sbb = _bcast_mid(sinh[:, t * 16:(t + 1) * 16], 8)
                    ts4 = []
                    for i in range(4):
                        ti = sb.tile([128, 8, 16], BF16, name=f"t{i}_{b}_{g}_{t}",
                                     tag=f"rtmp{i}", bufs=2)
                        ts4.append(ti)
                    nc.vector.tensor_tensor(ts4[0][:], ev, cb, mul)
                    nc.vector.tensor_tensor(ts4[1][:], od, sbb, mul)
                    nc.vector.tensor_tensor(ts4[2][:], od, cb, mul)
                    nc.vector.tensor_tensor(ts4[3][:], ev, sbb, mul)
                    nc.vector.tensor_tensor(ev, ts4[0][:], ts4[1][:], sub)
                    nc.vector.tensor_tensor(od, ts4[2][:], ts4[3][:], add)
                    # transpose [128 tok, 512 feat] -> 4 x [128 feat, 128 tok]
                    dst_view = (dst[:].rearrange("p (j c) -> p j c", c=N)
                                [:, :, t * 128:(t + 1) * 128])
                    nc.sync.dma_start_transpose(dst_view, qn[:])
                return eat

            def gen_qkv(b, j0, consume):
                """512-wide qkv chunk, one yield per matmul (64 total)."""
                wts = []
                for h in range(4):
                    w = sb.tile([128, 1024], BF16, name=f"wq_{b}_{j0}_{h}",
                                tag=f"wqh{h}", bufs=2)
                    eng = nc.sync if h % 2 == 0 else nc.scalar
                    eng.dma_start(
                        w[:].rearrange("p (d c) -> p d c", c=512),
                        wqkvT_d.ap()[h * 256:(h + 1) * 256, j0:j0 + 512]
                        .rearrange("(d p) c -> p d c", p=128))
                    wts.append(w)
                for t in range(NT):
                    pq = ps.tile([128, 512], F32, name=f"pq_{b}_{j0}_{t}",
                                 tag="pq", bufs=2)
                    for d in range(ND):
                        nc.tensor.matmul(
                            pq[:], xT[b][d][:, t * 128:(t + 1) * 128],
                            wts[d // 2][:, (d % 2) * 512:(d % 2 + 1) * 512],
                            start=(d == 0), stop=(d == ND - 1))
                        if d == ND - 1:
                            consume(t, pq)
                        yield

            def gen_proj(b, has_bias=HAS_BIAS):
                for t in range(NT):
                    for ec in range(2):
                        py = ps.tile([128, 512], F32, name=f"py_{b}_{t}_{ec}",
                                     tag="pq", bufs=2)
                        for d in range(ND):
                            nc.tensor.matmul(
                                py[:], outT[b][d][:, t * 128:(t + 1) * 128],
                                wprojT[d][:, ec * 512:(ec + 1) * 512],
                                start=(d == 0), stop=(d == ND - 1))
                            yield
                        ysb = sb.tile([128, 512], BF16, name=f"y_{b}_{t}_{ec}",
                                      tag="ysb", bufs=3)
                        if has_bias:
                            nc.vector.tensor_tensor(
                                ysb[:], py[:],
                                biasBC[:, ec * 512:(ec + 1) * 512], add)
                        else:
                            nc.vector.tensor_copy(ysb[:], py[:])
                        nc.sync.dma_start(
                            y_d.ap()[b, t * 128:(t + 1) * 128,
                                     ec * 512:(ec + 1) * 512], ysb[:])
                        yield

            # ---------------- attention ----------------
            def attention(b, g, filler, quota):
                credit = 0.0

                def fill():
                    nonlocal credit
                    credit += quota
                    k = int(credit)
                    credit -= k
                    _drain(filler, k)

                qt, kt = qTg[(b, g)], kTg[(b, g)]
                for hl in range(4):
                    hp = 4 * g + hl
                    for nch in range(2):
                        pvs = [ps.tile([HD + 1, 512], F32,
                                       name=f"pv_{b}_{hp}_{nch}_{i}",
                                       tag="pv", bufs=2) for i in range(2)]
                        pts = {}
                        for m in range(NT + 2):
                            if m < NT:
                                st2 = ps.tile([128, 1024], F32,
                                              name=f"st_{b}_{hp}_{nch}_{m}",
                                              tag="st2", bufs=2)
                                for half in range(2):
                                    r0 = half * 64
                                    nc.tensor.matmul(
                                        st2[:, half * 512:(half + 1) * 512],
                                        kt[r0:r0 + 64,
                                           hl * N + m * 128: hl * N + (m + 1) * 128],
                                        qt[r0:r0 + 64,
                                           hl * N + nch * 512: hl * N + nch * 512 + 512])
                                pt2 = sb.tile([128, 1024], BF16,
                                              name=f"pt_{b}_{hp}_{nch}_{m}",
                                              tag="pt2", bufs=5)
                                nc.scalar.activation(pt2[:], st2[:], EXP,
                                                     scale=float(SCALE))
                                pts[m] = pt2
                            fill()
                            if m > 1:
                                pm = m - 2
                                for half in range(2):
                                    h = hp * 2 + half
                                    nc.tensor.matmul(
                                        pvs[half][:],
                                        vsb[b][pm][:, h * (HD + 1):(h + 1) * (HD + 1)],
                                        pts[pm][:, half * 512:(half + 1) * 512],
                                        start=(pm == 0), stop=(pm == NT - 1))
                                del pts[pm]
                            fill()
                        # normalize into outT; evacuate psum first (two quick
                        # copies) so the pv bank frees before the slow
                        # recip/broadcast part of the chain
                        for half in range(2):
                            p = pvs[half]
                            pvc = sb.tile([64, 512], BF16,
                                          name=f"pvc_{b}_{hp}_{nch}_{half}",
                                          tag="pvc", bufs=2)
                            nc.vector.tensor_copy(pvc[:], p[0:64, :])
                            dr = sb.tile([1, 512], F32,
                                         name=f"dr_{b}_{hp}_{nch}_{half}",
                                         tag="dr", bufs=2)
                            nc.vector.tensor_copy(dr[:], p[64:65, :])
                            rr = sb.tile([1, 512], F32,
                                         name=f"rr_{b}_{hp}_{nch}_{half}",
                                         tag="rr", bufs=2)
                            nc.vector.reciprocal_approx_fast(rr[:], dr[:])
                            rb = sb.tile([64, 512], F32,
                                         name=f"rb_{b}_{hp}_{nch}_{half}",
                                         tag="rb", bufs=2)
                            nc.gpsimd.partition_broadcast(rb[:], rr[:])
                            nc.vector.tensor_tensor(
                                outT[b][hp][half * 64:half * 64 + 64,
                                            nch * 512:(nch + 1) * 512],
                                pvc[:], rb[:], mul)

            # ---------------- schedule ----------------
            QS, KS, VS = 0, DIM, 2 * DIM

            # Phase A: b0 V + b0 g0 K,Q (pure PE streams; warms HAM)
            # DMA order matters: per-queue transfers serialize, so emit only
            # xT d0/d1 before the first chunk's weight loads.
            alloc_x(0, ds=[0, 1])
            alloc_v(0)
            alloc_out(0)
            gv0 = gen_qkv(0, VS, eat_v(0, 0))
            _drain(gv0, 1)          # emits the chunk's weight DMAs + first MM
            alloc_x(0, ds=[2, 3, 4, 5, 6, 7])
            _drain_all(gv0)
            alloc_qk(0, 0)
            _drain_all(gen_qkv(0, KS + 0 * 512, eat_qk(0, 0, kTg[(0, 0)])))
            _drain_all(gen_qkv(0, QS + 0 * 512, eat_qk(0, 0, qTg[(0, 0)])))

            # Phase B: attn(b0,g0) + filler K,Q(b0,g1)
            alloc_x(1)
            alloc_qk(0, 1)
            fB = _chain(gen_qkv(0, KS + 512, eat_qk(0, 1, kTg[(0, 1)])),
                        gen_qkv(0, QS + 512, eat_qk(0, 1, qTg[(0, 1)])),
                        gen_qkv(0, VS + 512, eat_v(0, 1)))
            attention(0, 0, fB, 192 / 160)
            _drain_all(fB)

            # Phase C: attn(b0,g1) + filler V(b1), K,Q(b1,g0)
            load_wproj()
            alloc_v(1)
            alloc_out(1)
            alloc_qk(1, 0)
            fC = _chain(gen_qkv(1, VS, eat_v(1, 0)),
                        gen_qkv(1, KS, eat_qk(1, 0, kTg[(1, 0)])),
                        gen_qkv(1, QS, eat_qk(1, 0, qTg[(1, 0)])))
            attention(0, 1, fC, 192 / 160)
            _drain_all(fC)

            # Phase D: attn(b1,g0) + filler K,Q(b1,g1)
            alloc_qk(1, 1)
            fD = _chain(gen_qkv(1, KS + 512, eat_qk(1, 1, kTg[(1, 1)])),
                        gen_qkv(1, QS + 512, eat_qk(1, 1, qTg[(1, 1)])),
                        gen_qkv(1, VS + 512, eat_v(1, 1)))
            attention(1, 0, fD, 192 / 160)
            _drain_all(fD)

            # Phase E: attn(b1,g1) + filler proj(b0)
            fE = gen_proj(0)
            attention(1, 1, fE, 144 / 160)
            _drain_all(fE)

            # Phase F: proj(b1)
            _drain_all(gen_proj(1))

    nc.compile()
    return nc


_NC_CACHE = None


def _head_perm():
    """Per-head de-interleave: [evens of rot, odds of rot, pass dims]."""
    p = list(range(0, ROT, 2)) + list(range(1, ROT, 2)) + list(range(ROT, HD))
    return np.array(p)


def kernel(x, w_qkv, w_proj, b_proj):
    global _NC_CACHE, last_exec_time_ns
    import ml_dtypes
    bf = ml_dtypes.bfloat16
    x = np.ascontiguousarray(np.asarray(x, np.float32))
    w_qkv = np.asarray(w_qkv, np.float32)
    w_proj = np.asarray(w_proj, np.float32)
    b_proj = np.asarray(b_proj, np.float32)

    if _NC_CACHE is None:
        _NC_CACHE = _build(has_bias=bool(np.any(b_proj != 0)))
    nc = _NC_CACHE

    cos_h, sin_h = _freq_tables()
    # reorder q/k rows: per head [evens|odds|pass]
    perm = _head_perm()
    full_perm = np.concatenate([h * HD + perm for h in range(NH)])
    wq = w_qkv[:DIM][full_perm]
    wk = w_qkv[DIM:2 * DIM][full_perm]
    wv = w_qkv[2 * DIM:]
    wqkvT = np.ascontiguousarray(
        np.concatenate([wq, wk, wv], axis=0).T).astype(bf)
    wprojT16 = np.ascontiguousarray(w_proj.T).astype(bf)
    biasbc16 = np.ascontiguousarray(
        np.broadcast_to(b_proj.reshape(1, DIM), (128, DIM))).astype(bf)
    cos16 = cos_h.astype(bf)
    sin16 = sin_h.astype(bf)

    in_maps = []
    for c in range(NCORES):
        xs = x[c * BPC:(c + 1) * BPC]                       # [2, N, DIM]
        xT = np.ascontiguousarray(xs.transpose(0, 2, 1)).astype(bf)
        in_maps.append({
            "xT": xT, "wqkvT": wqkvT, "wprojT": wprojT16,
            "biasbc": biasbc16, "cosh": cos16, "sinh": sin16,
        })

    trace = bool(os.environ.get("KERNEL_TRACE"))
    kwargs = {}
    if trace:
        kwargs["trace"] = True
        td = os.environ.get("KERNEL_TRACE_DIR")
        if td:
            kwargs["tmpdir"] = td
    res = bass_utils.run_bass_kernel_spmd(
        nc, in_maps, core_ids=list(range(NCORES)), **kwargs)
    last_exec_time_ns = res.exec_time_ns
    out = np.concatenate(
        [np.asarray(res.results[c]["y"]) for c in range(NCORES)], axis=0)
    return np.ascontiguousarray(
        out.reshape(B, N, DIM).astype(np.float32))


if __name__ == "__main__":
    rng = np.random.default_rng(0)
    xs = rng.standard_normal((B, N, DIM), dtype=np.float32)
    wq = rng.standard_normal((3 * DIM, DIM), dtype=np.float32) / 32
    wp = rng.standard_normal((DIM, DIM), dtype=np.float32) / 32
    bp = np.zeros(DIM, np.float32)
    y = kernel(xs, wq, wp, bp)
    print("y", y.shape, y.dtype, float(np.abs(y).max()))
